# revision 16
# baseline (speedup 1.0000x reference)
"""Trainium2 Bass kernel for nn_DWTExtractor: 2-level Haar DWT + bilinear 2x upsample.

Input  x: (32, 1, 1024, 1024) fp32
Output y: (32, 6, 512, 512) fp32 = [cH1, cV1, cD1, cH2u, cV2u, cD2u]

Sharding: pure batch data-parallel, 4 images per core across 8 cores.

Per-core dataflow (per image, 4 chunks of 256 input rows):
  - PE (fp32r) computes all cross-row (H-direction) work as banded matmuls:
      L1/L2 Haar row-pairing (+-0.5 weights baked in) and the bilinear
      H-upsample (0.75/0.25 taps, x0.25 de-scale folded in).
  - ACT evacuates even-index columns of PSUM (strided copies).
  - DVE does the W-direction pair sum/diff as tensor_tensor with one SBUF
    (evacuated evens) and one strided PSUM (odds) operand.
  - GPSIMD does the W-direction bilinear upsample: t3 = 3*band, then
    out_even = t3 + band[j-1], out_odd = t3 + band[j+1] (values are 4x the
    true upsample; the 1/4 is folded into the H-upsample matrices).
"""

import numpy as np

import concourse.bass as bass
import concourse.tile as tile
import concourse.mybir as mybir
from concourse import bacc, bass_utils

F32 = mybir.dt.float32
F32R = mybir.dt.float32r
AL = mybir.AluOpType

B, H, W = 32, 1024, 1024
NCORES = 8
IMG = B // NCORES  # images per core
HL, WL = H // 2, W // 2  # 512, 512 (level-1 band size)
H2, W2 = H // 4, W // 4  # 256, 256 (level-2 band size)
P = 128


def _build_const_matrix() -> np.ndarray:
    """(128, 10*128) fp32: PS_lo|PS_hi|PD_lo|PD_hi|U0|U1|U2|U3|U1b|U2b."""
    ps_lo = np.zeros((P, P), np.float32)
    ps_hi = np.zeros((P, P), np.float32)
    pd_lo = np.zeros((P, P), np.float32)
    pd_hi = np.zeros((P, P), np.float32)
    for i in range(64):
        ps_lo[2 * i, i] = 0.5
        ps_lo[2 * i + 1, i] = 0.5
        ps_hi[2 * i, 64 + i] = 0.5
        ps_hi[2 * i + 1, 64 + i] = 0.5
        pd_lo[2 * i, i] = 0.5
        pd_lo[2 * i + 1, i] = -0.5
        pd_hi[2 * i, 64 + i] = 0.5
        pd_hi[2 * i + 1, 64 + i] = -0.5

    # H-upsample matrix (256 src rows -> 512 out rows), half-pixel bilinear
    # with edge clamp; x0.25 folded in (wup values are 4x true).
    u_full = np.zeros((H2, HL), np.float32)
    for m in range(HL):
        k = m // 2
        if m % 2 == 0:
            taps = [(k, 0.75), (k - 1, 0.25)]
        else:
            taps = [(k, 0.75), (k + 1, 0.25)]
        for src, wgt in taps:
            u_full[min(max(src, 0), H2 - 1), m] += wgt
    u_full *= 0.25

    u0 = u_full[0:128, 0:128]
    u1 = u_full[0:128, 128:256]
    u2 = u_full[128:256, 256:384]
    u3 = u_full[128:256, 384:512]
    u1b = np.zeros((P, P), np.float32)
    u1b[0, :] = u_full[128, 128:256]
    u2b = np.zeros((P, P), np.float32)
    u2b[127, :] = u_full[127, 256:384]

    return np.concatenate(
        [ps_lo, ps_hi, pd_lo, pd_hi, u0, u1, u2, u3, u1b, u2b], axis=1
    )


def build_nc() -> "bacc.Bacc":
    nc = bacc.Bacc(
        "TRN2", target_bir_lowering=False, debug=False, num_devices=NCORES,
        name="dwt_extractor",
    )
    x_d = nc.dram_tensor("xc", [IMG, H, W], F32R, kind="ExternalInput")
    wm_d = nc.dram_tensor("wm", [P, 10 * P], F32R, kind="ExternalInput")
    y_d = nc.dram_tensor("yc", [IMG, 6, HL, WL], F32, kind="ExternalOutput")

    with tile.TileContext(nc) as tc:
        with (
            tc.tile_pool(name="consts", bufs=1) as cpool,
            tc.tile_pool(name="xin", bufs=4) as xpool,
            tc.tile_pool(name="ev", bufs=4) as evpool,
            tc.tile_pool(name="ca1", bufs=3) as ca1pool,
            tc.tile_pool(name="bands3", bufs=3) as b3pool,
            tc.tile_pool(name="t3", bufs=2) as t3pool,
            tc.tile_pool(name="wup3", bufs=4) as wuppool,
            tc.tile_pool(name="stg", bufs=2) as stgpool,
            tc.tile_pool(name="psS", bufs=1, space="PSUM") as psS,
            tc.tile_pool(name="psD", bufs=1, space="PSUM") as psD,
            tc.tile_pool(name="psL2", bufs=1, space="PSUM") as psL2,
            tc.tile_pool(name="psUp", bufs=2, space="PSUM") as psUp,
        ):
            wm = cpool.tile([P, 10 * P], F32R)
            nc.sync.dma_start(wm[:], wm_d[:])
            blk = lambda i: wm[:, i * P : (i + 1) * P]
            PS_lo, PS_hi, PD_lo, PD_hi = blk(0), blk(1), blk(2), blk(3)
            U0, U1, U2, U3 = blk(4), blk(5), blk(6), blk(7)
            U1b = blk(8)
            U2b = blk(9)

            def stage_a_chunk(b, u, ca1, stgL1):
                if True:
                    xu = xpool.tile([P, 2048], F32R, tag="x")
                    src = x_d[b, 256 * u : 256 * (u + 1), :]
                    nc.sync.dma_start(
                        xu[:].rearrange("p (t w) -> p t w", t=2),
                        src.rearrange("(t p) w -> p t w", t=2),
                    )
                    sS = psS.tile([P, 1024], F32, tag="S")
                    sD = psD.tile([P, 1024], F32, tag="D")
                    for h in range(2):
                        o = 512 * h
                        nc.tensor.matmul(
                            sS[:, o : o + 512], PS_lo, xu[:, o : o + 512],
                            start=True, stop=False,
                        )
                        nc.tensor.matmul(
                            sS[:, o : o + 512], PS_hi, xu[:, 1024 + o : 1536 + o],
                            start=False, stop=True,
                        )
                    for h in range(2):
                        o = 512 * h
                        nc.tensor.matmul(
                            sD[:, o : o + 512], PD_lo, xu[:, o : o + 512],
                            start=True, stop=False,
                        )
                        nc.tensor.matmul(
                            sD[:, o : o + 512], PD_hi, xu[:, 1024 + o : 1536 + o],
                            start=False, stop=True,
                        )

                    se = evpool.tile([P, 512], F32, tag="se")
                    de = evpool.tile([P, 512], F32, tag="se")
                    nc.scalar.copy(se[:], sS[:, 0:1024:2])
                    nc.scalar.copy(de[:], sD[:, 0:1024:2])

                    cu = ca1pool.tile([P, 512], F32R, tag="ca1")
                    nc.vector.tensor_tensor(cu[:], se[:], sS[:, 1:1024:2], AL.add)
                    ca1.append(cu)
                    if u == 0:
                        stgH1 = stgpool.tile([P, 2048], F32, tag="sH1")
                        stgV1 = stgpool.tile([P, 2048], F32, tag="sV1")
                        stgD1 = stgpool.tile([P, 2048], F32, tag="sD1")
                        stgL1.extend([stgH1, stgV1, stgD1])
                    o = 512 * u
                    nc.vector.tensor_tensor(
                        stgL1[0][:, o : o + 512], se[:], sS[:, 1:1024:2], AL.subtract
                    )
                    nc.vector.tensor_tensor(
                        stgL1[1][:, o : o + 512], de[:], sD[:, 1:1024:2], AL.add
                    )
                    nc.vector.tensor_tensor(
                        stgL1[2][:, o : o + 512], de[:], sD[:, 1:1024:2], AL.subtract
                    )

            def stage_a_pair(b, v, ca1):
                if True:
                    sd2 = psL2.tile([P, 1024], F32, tag="sd2")
                    nc.tensor.matmul(
                        sd2[:, 0:512], PS_lo, ca1[2 * v][:], start=True, stop=False
                    )
                    nc.tensor.matmul(
                        sd2[:, 0:512], PS_hi, ca1[2 * v + 1][:], start=False, stop=True
                    )
                    nc.tensor.matmul(
                        sd2[:, 512:1024], PD_lo, ca1[2 * v][:], start=True, stop=False
                    )
                    nc.tensor.matmul(
                        sd2[:, 512:1024], PD_hi, ca1[2 * v + 1][:], start=False, stop=True
                    )

                    s2e = evpool.tile([P, 256], F32, tag="s2e")
                    d2e = evpool.tile([P, 256], F32, tag="s2e")
                    nc.scalar.copy(s2e[:], sd2[:, 0:512:2])
                    nc.scalar.copy(d2e[:], sd2[:, 512:1024:2])

                    specs = [
                        (s2e, sd2[:, 1:512:2], AL.subtract),  # cH2
                        (d2e, sd2[:, 513:1024:2], AL.add),  # cV2
                        (d2e, sd2[:, 513:1024:2], AL.subtract),  # cD2
                    ]
                    b3 = b3pool.tile([P, 768], F32, tag="b3")
                    for band, (ev, od, op) in enumerate(specs):
                        nc.vector.tensor_tensor(
                            b3[:, 256 * band : 256 * (band + 1)], ev[:], od, op
                        )
                    t3 = t3pool.tile([P, 768], F32, tag="t3")
                    nc.vector.tensor_scalar_mul(t3[:], b3[:], 3.0)
                    wu = wuppool.tile([P, 1536], F32R, tag="wup")
                    wu_r = wu[:].rearrange("p (b w) -> p b w", b=3)
                    b3_r = b3[:].rearrange("p (b w) -> p b w", b=3)
                    t3_r = t3[:].rearrange("p (b w) -> p b w", b=3)
                    nc.gpsimd.tensor_tensor(
                        wu_r[:, :, 2:512:2], t3_r[:, :, 1:256], b3_r[:, :, 0:255], AL.add
                    )
                    nc.gpsimd.tensor_tensor(
                        wu_r[:, :, 1:511:2], t3_r[:, :, 0:255], b3_r[:, :, 1:256], AL.add
                    )
                    nc.vector.tensor_scalar_mul(
                        wu_r[:, :, 0:512:511], b3_r[:, :, 0:256:255], 4.0
                    )
                    return wu

            def emit_l1_outs(b, stgL1):
                for band in range(3):
                    dst = y_d[b, band]
                    nc.sync.dma_start(
                        dst.rearrange("(u p) w -> p u w", u=4),
                        stgL1[band][:].rearrange("p (u w) -> p u w", u=4),
                    )

            def stage_b_band(b, wup3s, band):
                if True:
                    w0 = wup3s[0][:, 512 * band : 512 * (band + 1)]
                    w1 = wup3s[1][:, 512 * band : 512 * (band + 1)]
                    st = stgpool.tile([P, 2048], F32, tag=f"s2b{band}")
                    up = psUp.tile([P, 512], F32, tag="up")
                    nc.tensor.matmul(up[:], U0, w0, start=True, stop=True)
                    nc.scalar.copy(st[:, 0:512], up[:])
                    up = psUp.tile([P, 512], F32, tag="up")
                    nc.tensor.matmul(up[:], U1, w0, start=True, stop=False)
                    nc.tensor.matmul(up[:], U1b, w1, start=False, stop=True)
                    nc.scalar.copy(st[:, 512:1024], up[:])
                    up = psUp.tile([P, 512], F32, tag="up")
                    nc.tensor.matmul(up[:], U2, w1, start=True, stop=False)
                    nc.tensor.matmul(up[:], U2b, w0, start=False, stop=True)
                    nc.scalar.copy(st[:, 1024:1536], up[:])
                    up = psUp.tile([P, 512], F32, tag="up")
                    nc.tensor.matmul(up[:], U3, w1, start=True, stop=True)
                    nc.scalar.copy(st[:, 1536:2048], up[:])
                    dst = y_d[b, 3 + band]
                    nc.sync.dma_start(
                        dst.rearrange("(u p) w -> p u w", u=4),
                        st[:].rearrange("p (u w) -> p u w", u=4),
                    )

            pending = None
            for b in range(IMG):
                ca1 = []
                stgL1 = []
                wup3s = [None, None]
                stage_a_chunk(b, 0, ca1, stgL1)
                if pending is not None:
                    stage_b_band(pending[0], pending[1], 0)
                stage_a_chunk(b, 1, ca1, stgL1)
                wup3s[0] = stage_a_pair(b, 0, ca1)
                if pending is not None:
                    stage_b_band(pending[0], pending[1], 1)
                stage_a_chunk(b, 2, ca1, stgL1)
                if pending is not None:
                    stage_b_band(pending[0], pending[1], 2)
                stage_a_chunk(b, 3, ca1, stgL1)
                wup3s[1] = stage_a_pair(b, 1, ca1)
                emit_l1_outs(b, stgL1)
                pending = (b, wup3s)
            for band in range(3):
                stage_b_band(pending[0], pending[1], band)

    nc.compile()
    return nc


_NC_CACHE = None
LAST_RESULTS = None


def kernel(**inputs) -> np.ndarray:
    global _NC_CACHE, LAST_RESULTS
    trace = bool(inputs.pop("_trace", False))
    x = np.ascontiguousarray(np.asarray(inputs["x"], dtype=np.float32))
    assert x.shape == (B, 1, H, W), x.shape
    if _NC_CACHE is None:
        _NC_CACHE = build_nc()
    nc = _NC_CACHE
    wm = _build_const_matrix()
    in_maps = [
        {"xc": np.ascontiguousarray(x[IMG * c : IMG * (c + 1), 0]), "wm": wm}
        for c in range(NCORES)
    ]
    res = bass_utils.run_bass_kernel_spmd(
        nc, in_maps, core_ids=list(range(NCORES)), trace=trace
    )
    LAST_RESULTS = res
    out = np.concatenate([res.results[c]["yc"] for c in range(NCORES)], axis=0)
    return out.astype(np.float32)


if __name__ == "__main__":
    rng = np.random.default_rng(0)
    x = rng.standard_normal((B, 1, H, W), dtype=np.float32)
    y = kernel(x=x)
    print("kernel output:", y.shape, y.dtype)


# revision 17
# speedup vs baseline: 1.0640x; 1.0640x over previous
"""Trainium2 Bass kernel for nn_DWTExtractor: 2-level Haar DWT + bilinear 2x upsample.

Input  x: (32, 1, 1024, 1024) fp32
Output y: (32, 6, 512, 512) fp32 = [cH1, cV1, cD1, cH2u, cV2u, cD2u]

Sharding: pure batch data-parallel, 4 images per core across 8 cores.

Per-core dataflow (per image, 4 chunks of 256 input rows):
  - PE (fp32r) computes all cross-row (H-direction) work as banded matmuls:
      L1/L2 Haar row-pairing (+-0.5 weights baked in) and the bilinear
      H-upsample (0.75/0.25 taps, x0.25 de-scale folded in).
  - ACT evacuates even-index columns of PSUM (strided copies).
  - DVE does the W-direction pair sum/diff as tensor_tensor with one SBUF
    (evacuated evens) and one strided PSUM (odds) operand.
  - GPSIMD does the W-direction bilinear upsample: t3 = 3*band, then
    out_even = t3 + band[j-1], out_odd = t3 + band[j+1] (values are 4x the
    true upsample; the 1/4 is folded into the H-upsample matrices).
"""

import numpy as np

import concourse.bass as bass
import concourse.tile as tile
import concourse.mybir as mybir
from concourse import bacc, bass_utils

F32 = mybir.dt.float32
F32R = mybir.dt.float32r
AL = mybir.AluOpType

B, H, W = 32, 1024, 1024
NCORES = 8
IMG = B // NCORES  # images per core
HL, WL = H // 2, W // 2  # 512, 512 (level-1 band size)
H2, W2 = H // 4, W // 4  # 256, 256 (level-2 band size)
P = 128


def _build_const_matrix() -> np.ndarray:
    """(128, 10*128) fp32: PS_lo|PS_hi|PD_lo|PD_hi|U0|U1|U2|U3|U1b|U2b."""
    ps_lo = np.zeros((P, P), np.float32)
    ps_hi = np.zeros((P, P), np.float32)
    pd_lo = np.zeros((P, P), np.float32)
    pd_hi = np.zeros((P, P), np.float32)
    for i in range(64):
        ps_lo[2 * i, i] = 0.5
        ps_lo[2 * i + 1, i] = 0.5
        ps_hi[2 * i, 64 + i] = 0.5
        ps_hi[2 * i + 1, 64 + i] = 0.5
        pd_lo[2 * i, i] = 0.5
        pd_lo[2 * i + 1, i] = -0.5
        pd_hi[2 * i, 64 + i] = 0.5
        pd_hi[2 * i + 1, 64 + i] = -0.5

    # H-upsample matrix (256 src rows -> 512 out rows), half-pixel bilinear
    # with edge clamp; x0.25 folded in (wup values are 4x true).
    u_full = np.zeros((H2, HL), np.float32)
    for m in range(HL):
        k = m // 2
        if m % 2 == 0:
            taps = [(k, 0.75), (k - 1, 0.25)]
        else:
            taps = [(k, 0.75), (k + 1, 0.25)]
        for src, wgt in taps:
            u_full[min(max(src, 0), H2 - 1), m] += wgt
    u_full *= 0.25

    u0 = u_full[0:128, 0:128]
    u1 = u_full[0:128, 128:256]
    u2 = u_full[128:256, 256:384]
    u3 = u_full[128:256, 384:512]
    u1b = np.zeros((P, P), np.float32)
    u1b[0, :] = u_full[128, 128:256]
    u2b = np.zeros((P, P), np.float32)
    u2b[127, :] = u_full[127, 256:384]

    return np.concatenate(
        [ps_lo, ps_hi, pd_lo, pd_hi, u0, u1, u2, u3, u1b, u2b], axis=1
    )


def build_nc() -> "bacc.Bacc":
    nc = bacc.Bacc(
        "TRN2", target_bir_lowering=False, debug=False, num_devices=NCORES,
        name="dwt_extractor",
    )
    x_d = nc.dram_tensor("xc", [IMG, H, W], F32R, kind="ExternalInput")
    wm_d = nc.dram_tensor("wm", [P, 10 * P], F32R, kind="ExternalInput")
    y_d = nc.dram_tensor("yc", [IMG, 6, HL, WL], F32, kind="ExternalOutput")

    with tile.TileContext(nc) as tc:
        with (
            tc.tile_pool(name="consts", bufs=1) as cpool,
            tc.tile_pool(name="xin", bufs=5) as xpool,
            tc.tile_pool(name="ev", bufs=6) as evpool,
            tc.tile_pool(name="ca1", bufs=4) as ca1pool,
            tc.tile_pool(name="bands3", bufs=4) as b3pool,
            tc.tile_pool(name="t3", bufs=3) as t3pool,
            tc.tile_pool(name="wup3", bufs=4) as wuppool,
            tc.tile_pool(name="stg", bufs=2) as stgpool,
            tc.tile_pool(name="stg2", bufs=1) as stg2pool,
            tc.tile_pool(name="psS", bufs=1, space="PSUM") as psS,
            tc.tile_pool(name="psD", bufs=1, space="PSUM") as psD,
            tc.tile_pool(name="psL2", bufs=1, space="PSUM") as psL2,
            tc.tile_pool(name="psUp", bufs=2, space="PSUM") as psUp,
        ):
            wm = cpool.tile([P, 10 * P], F32R)
            nc.sync.dma_start(wm[:], wm_d[:])
            blk = lambda i: wm[:, i * P : (i + 1) * P]
            PS_lo, PS_hi, PD_lo, PD_hi = blk(0), blk(1), blk(2), blk(3)
            U0, U1, U2, U3 = blk(4), blk(5), blk(6), blk(7)
            U1b = blk(8)
            U2b = blk(9)

            def stage_a(b):
                """L1 chunks + L2 + W-upsample for image b; returns wup3s."""
                ca1 = []
                stgL1 = []
                for u in range(4):
                    xu = xpool.tile([P, 2048], F32R, tag="x")
                    src = x_d[b, 256 * u : 256 * (u + 1), :]
                    nc.sync.dma_start(
                        xu[:].rearrange("p (t w) -> p t w", t=2),
                        src.rearrange("(t p) w -> p t w", t=2),
                    )
                    sS = psS.tile([P, 1024], F32, tag="S")
                    sD = psD.tile([P, 1024], F32, tag="D")
                    for h in range(2):
                        o = 512 * h
                        nc.tensor.matmul(
                            sS[:, o : o + 512], PS_lo, xu[:, o : o + 512],
                            start=True, stop=False,
                        )
                        nc.tensor.matmul(
                            sS[:, o : o + 512], PS_hi, xu[:, 1024 + o : 1536 + o],
                            start=False, stop=True,
                        )
                    for h in range(2):
                        o = 512 * h
                        nc.tensor.matmul(
                            sD[:, o : o + 512], PD_lo, xu[:, o : o + 512],
                            start=True, stop=False,
                        )
                        nc.tensor.matmul(
                            sD[:, o : o + 512], PD_hi, xu[:, 1024 + o : 1536 + o],
                            start=False, stop=True,
                        )

                    se = evpool.tile([P, 512], F32, tag="se")
                    de = evpool.tile([P, 512], F32, tag="se")
                    nc.scalar.copy(se[:], sS[:, 0:1024:2])
                    nc.scalar.copy(de[:], sD[:, 0:1024:2])

                    cu = ca1pool.tile([P, 512], F32R, tag="ca1")
                    nc.vector.tensor_tensor(cu[:], se[:], sS[:, 1:1024:2], AL.add)
                    ca1.append(cu)
                    if u == 0:
                        stgH1 = stgpool.tile([P, 2048], F32, tag="sH1")
                        stgV1 = stgpool.tile([P, 2048], F32, tag="sV1")
                        stgD1 = stgpool.tile([P, 2048], F32, tag="sD1")
                        stgL1 = [stgH1, stgV1, stgD1]
                    o = 512 * u
                    nc.vector.tensor_tensor(
                        stgL1[0][:, o : o + 512], se[:], sS[:, 1:1024:2], AL.subtract
                    )
                    nc.vector.tensor_tensor(
                        stgL1[1][:, o : o + 512], de[:], sD[:, 1:1024:2], AL.add
                    )
                    nc.vector.tensor_tensor(
                        stgL1[2][:, o : o + 512], de[:], sD[:, 1:1024:2], AL.subtract
                    )

                # level 2 + W-upsample; wup3s[v] = (128, 3*512) f32r
                wup3s = [None, None]
                for v in range(2):
                    sd2 = psL2.tile([P, 1024], F32, tag="sd2")
                    nc.tensor.matmul(
                        sd2[:, 0:512], PS_lo, ca1[2 * v][:], start=True, stop=False
                    )
                    nc.tensor.matmul(
                        sd2[:, 0:512], PS_hi, ca1[2 * v + 1][:], start=False, stop=True
                    )
                    nc.tensor.matmul(
                        sd2[:, 512:1024], PD_lo, ca1[2 * v][:], start=True, stop=False
                    )
                    nc.tensor.matmul(
                        sd2[:, 512:1024], PD_hi, ca1[2 * v + 1][:], start=False, stop=True
                    )

                    s2e = evpool.tile([P, 256], F32, tag="s2e")
                    d2e = evpool.tile([P, 256], F32, tag="s2e")
                    nc.scalar.copy(s2e[:], sd2[:, 0:512:2])
                    nc.scalar.copy(d2e[:], sd2[:, 512:1024:2])

                    specs = [
                        (s2e, sd2[:, 1:512:2], AL.subtract),  # cH2
                        (d2e, sd2[:, 513:1024:2], AL.add),  # cV2
                        (d2e, sd2[:, 513:1024:2], AL.subtract),  # cD2
                    ]
                    b3 = b3pool.tile([P, 768], F32, tag="b3")
                    for band, (ev, od, op) in enumerate(specs):
                        nc.vector.tensor_tensor(
                            b3[:, 256 * band : 256 * (band + 1)], ev[:], od, op
                        )
                    t3 = t3pool.tile([P, 768], F32, tag="t3")
                    nc.vector.tensor_scalar_mul(t3[:], b3[:], 3.0)
                    wu = wuppool.tile([P, 1536], F32R, tag="wup")
                    wu_r = wu[:].rearrange("p (b w) -> p b w", b=3)
                    b3_r = b3[:].rearrange("p (b w) -> p b w", b=3)
                    t3_r = t3[:].rearrange("p (b w) -> p b w", b=3)
                    nc.gpsimd.tensor_tensor(
                        wu_r[:, :, 2:512:2], t3_r[:, :, 1:256], b3_r[:, :, 0:255], AL.add
                    )
                    nc.gpsimd.tensor_tensor(
                        wu_r[:, :, 1:511:2], t3_r[:, :, 0:255], b3_r[:, :, 1:256], AL.add
                    )
                    nc.vector.tensor_scalar_mul(
                        wu_r[:, :, 0:512:511], b3_r[:, :, 0:256:255], 4.0
                    )
                    wup3s[v] = wu

                # L1 band outputs can stream out now
                for band in range(3):
                    dst = y_d[b, band]
                    nc.sync.dma_start(
                        dst.rearrange("(u p) w -> p u w", u=4),
                        stgL1[band][:].rearrange("p (u w) -> p u w", u=4),
                    )
                return wup3s

            def stage_b(b, wup3s):
                """H-upsample + evacuation + upsampled-band outputs for image b."""
                for band in range(3):
                    w0 = wup3s[0][:, 512 * band : 512 * (band + 1)]
                    w1 = wup3s[1][:, 512 * band : 512 * (band + 1)]
                    st = stg2pool.tile([P, 2048], F32, tag=f"s2b{band}")
                    up = psUp.tile([P, 512], F32, tag="up")
                    nc.tensor.matmul(up[:], U0, w0, start=True, stop=True)
                    nc.scalar.copy(st[:, 0:512], up[:])
                    up = psUp.tile([P, 512], F32, tag="up")
                    nc.tensor.matmul(up[:], U1, w0, start=True, stop=False)
                    nc.tensor.matmul(up[:], U1b, w1, start=False, stop=True)
                    nc.scalar.copy(st[:, 512:1024], up[:])
                    up = psUp.tile([P, 512], F32, tag="up")
                    nc.tensor.matmul(up[:], U2, w1, start=True, stop=False)
                    nc.tensor.matmul(up[:], U2b, w0, start=False, stop=True)
                    nc.scalar.copy(st[:, 1024:1536], up[:])
                    up = psUp.tile([P, 512], F32, tag="up")
                    nc.tensor.matmul(up[:], U3, w1, start=True, stop=True)
                    nc.scalar.copy(st[:, 1536:2048], up[:])
                    dst = y_d[b, 3 + band]
                    nc.sync.dma_start(
                        dst.rearrange("(u p) w -> p u w", u=4),
                        st[:].rearrange("p (u w) -> p u w", u=4),
                    )

            pending = None
            for b in range(IMG):
                wup3s = stage_a(b)
                if pending is not None:
                    stage_b(pending[0], pending[1])
                pending = (b, wup3s)
            stage_b(pending[0], pending[1])

    nc.compile()
    return nc


_NC_CACHE = None
LAST_RESULTS = None


def kernel(**inputs) -> np.ndarray:
    global _NC_CACHE, LAST_RESULTS
    trace = bool(inputs.pop("_trace", False))
    x = np.ascontiguousarray(np.asarray(inputs["x"], dtype=np.float32))
    assert x.shape == (B, 1, H, W), x.shape
    if _NC_CACHE is None:
        _NC_CACHE = build_nc()
    nc = _NC_CACHE
    wm = _build_const_matrix()
    in_maps = [
        {"xc": np.ascontiguousarray(x[IMG * c : IMG * (c + 1), 0]), "wm": wm}
        for c in range(NCORES)
    ]
    res = bass_utils.run_bass_kernel_spmd(
        nc, in_maps, core_ids=list(range(NCORES)), trace=trace
    )
    LAST_RESULTS = res
    out = np.concatenate([res.results[c]["yc"] for c in range(NCORES)], axis=0)
    return out.astype(np.float32)


if __name__ == "__main__":
    rng = np.random.default_rng(0)
    x = rng.standard_normal((B, 1, H, W), dtype=np.float32)
    y = kernel(x=x)
    print("kernel output:", y.shape, y.dtype)
